# revision 7
# baseline (speedup 1.0000x reference)
"""Trainium2 Bass kernel for nn_AttentionFusion_13889924235277.

Data-parallel over batch: 32 samples -> 4 per NeuronCore x 8 cores.
Per-core SPMD program (Tile framework), bf16 compute via SWDGE DMA-cast:

  Q^T = Wq^T.. projections produce Q^T,K^T [hd,n] and V [m,hd] directly
  scores = Q K^T / sqrt(hd)            (TensorE, f32 PSUM)
  P = cw * exp(scores)                 (softmax(s + log cw) without log/max:
                                        scores ~ N(0,1), exp never overflows)
  l = row-sums via ones-column on V    (TensorE)
  attn = P / l                         (DVE, written f32 via DMA-cast)
  attended = (P^T)^T V / l             (PE transpose of P + TensorE)
  pooled means via free-axis reduce (cf^T) and ones-matmul (attended)
  head: BN folded into Wf1/bf1 on host; biases folded into per-partition
  evictions; out written transposed [OD, BL] and un-transposed on host.

Host-side marshaling: batch shard, cf/gf transposed to feature-major,
BN fold, bias reshapes, mean 1/N folded into Wcp/Wgp.
"""
import sys
import numpy as np

for _p in ("/opt/trn_rl_repo",):
    if _p not in sys.path:
        sys.path.append(_p)

B, N = 32, 1024
CD, GD, HD, OD = 896, 384, 256, 1280
EPS_BN = 1e-5
N_CORES = 8
BL = B // N_CORES  # 4 samples per core
P = 128
KC, KG = CD // P, GD // P          # 7, 3 contraction chunks
NH = HD // P                       # 2 hd chunks
NT = N // P                        # 8 row tiles
OC = OD // P                       # 10 od chunks
O2 = (OD // 2) // P                # 5 od/2 chunks

_CACHE = {}

# This walrus build rejects >1 sync-wait per instruction; Tile emits
# multi-wait drains. Split them onto chained same-engine NOPs.
def _split_waits(nc, mybir):
    f = nc.m.functions[0]
    for bb in f.blocks:
        newlist, changed = [], False
        for inst in bb.instructions:
            si = inst.sync_info
            waits = list(si.on_wait) if si and si.on_wait else []
            if len(waits) > 1:
                for w in waits[:-1]:
                    newlist.append(mybir.InstNoOp(
                        name=f"I-waitsplit-{nc.next_id()}",
                        engine=inst.engine, ins=[], outs=[],
                        sync_info=mybir.SyncInfo(on_wait=[w], on_update=[]),
                    ))
                inst.sync_info = mybir.SyncInfo(
                    on_wait=[waits[-1]], on_update=list(si.on_update or []))
                changed = True
            newlist.append(inst)
        if changed:
            bb.instructions = newlist


def _build(split=True, n_samples=BL, do_pv=True, do_head=True, do_attn_out=True):
    import concourse.bass as bass
    import concourse.tile as tile
    from concourse import mybir

    f32, bf16 = mybir.dt.float32, mybir.dt.bfloat16
    AF = mybir.ActivationFunctionType

    nc = bass.Bass()
    dp = nc.declare_dram_parameter
    cft_e = dp("cft", [BL, CD, N], f32, isOutput=False)
    gft_e = dp("gft", [BL, GD, N], f32, isOutput=False)
    cw_e = dp("cw", [BL, N, N], f32, isOutput=False)
    wq_e = dp("wq", [CD, HD], f32, isOutput=False)
    wk_e = dp("wk", [GD, HD], f32, isOutput=False)
    wv_e = dp("wv", [GD, HD], f32, isOutput=False)
    wcp_e = dp("wcp", [CD, OD // 2], f32, isOutput=False)
    wgp_e = dp("wgp", [HD, OD // 2], f32, isOutput=False)
    wf1_e = dp("wf1", [OD, OD], f32, isOutput=False)
    wf2_e = dp("wf2", [OD, OD], f32, isOutput=False)
    bqk_e = dp("bqk", [P, 2 * NH], f32, isOutput=False)   # bq|bk per-partition
    bv_e = dp("bv", [1, HD], f32, isOutput=False)
    bcomb_e = dp("bcomb", [P, OC], f32, isOutput=False)   # bcp|bgp
    bf1_e = dp("bf1", [P, OC], f32, isOutput=False)       # BN-folded
    bf2_e = dp("bf2", [P, OC], f32, isOutput=False)
    ident_e = dp("ident", [P, P], f32, isOutput=False)
    attn_e = dp("attn", [BL, N, N], f32, isOutput=True)
    outt_e = dp("outt", [OD, BL], f32, isOutput=True)

    with tile.TileContext(nc) as tc:
        import contextlib
        with contextlib.ExitStack() as ctx:
            consts = ctx.enter_context(tc.tile_pool(name="consts", bufs=1))
            a_pool = ctx.enter_context(tc.tile_pool(name="a", bufs=2))
            g_pool = ctx.enter_context(tc.tile_pool(name="g", bufs=1))
            qk_pool = ctx.enter_context(tc.tile_pool(name="qk", bufs=1))
            v_pool = ctx.enter_context(tc.tile_pool(name="v", bufs=1))
            cw_pool = ctx.enter_context(tc.tile_pool(name="cwp", bufs=3))
            e_pool = ctx.enter_context(tc.tile_pool(name="e", bufs=2))
            p_pool = ctx.enter_context(tc.tile_pool(name="p", bufs=2))
            pt_pool = ctx.enter_context(tc.tile_pool(name="pt", bufs=1))
            att_pool = ctx.enter_context(tc.tile_pool(name="att", bufs=1))
            small = ctx.enter_context(tc.tile_pool(name="small", bufs=1))
            ps_sc = ctx.enter_context(tc.tile_pool(name="ps_sc", bufs=2, space="PSUM"))
            ps_pj = ctx.enter_context(tc.tile_pool(name="ps_pj", bufs=1, space="PSUM"))
            ps_pt = ctx.enter_context(tc.tile_pool(name="ps_pt", bufs=1, space="PSUM"))
            ps_av = ctx.enter_context(tc.tile_pool(name="ps_av", bufs=1, space="PSUM"))
            ps_gp = ctx.enter_context(tc.tile_pool(name="ps_gp", bufs=1, space="PSUM"))

            # ---- constants / weights (DMA-cast f32 -> bf16) ----
            wq = consts.tile([P, KC, HD], bf16)
            nc.gpsimd.dma_start(out=wq[:], in_=wq_e.rearrange("(k p) h -> p k h", p=P))
            wk = consts.tile([P, KG, HD], bf16)
            nc.gpsimd.dma_start(out=wk[:], in_=wk_e.rearrange("(k p) h -> p k h", p=P))
            wv = consts.tile([P, KG, HD], bf16)
            nc.gpsimd.dma_start(out=wv[:], in_=wv_e.rearrange("(k p) h -> p k h", p=P))
            wcp = consts.tile([P, KC, OD // 2], bf16)
            nc.gpsimd.dma_start(out=wcp[:], in_=wcp_e.rearrange("(k p) h -> p k h", p=P))
            wgp = consts.tile([P, NH, OD // 2], bf16)
            nc.gpsimd.dma_start(out=wgp[:], in_=wgp_e.rearrange("(k p) h -> p k h", p=P))
            wf1 = consts.tile([P, OC, OD], bf16)
            nc.gpsimd.dma_start(out=wf1[:], in_=wf1_e.rearrange("(k p) h -> p k h", p=P))
            wf2 = consts.tile([P, OC, OD], bf16)
            nc.gpsimd.dma_start(out=wf2[:], in_=wf2_e.rearrange("(k p) h -> p k h", p=P))
            bqk = consts.tile([P, 2 * NH], f32)
            nc.gpsimd.dma_start(out=bqk[:], in_=bqk_e[:])
            bv = consts.tile([1, HD], bf16)
            nc.gpsimd.dma_start(out=bv[:], in_=bv_e[:])
            bcomb = consts.tile([P, OC], f32)
            nc.gpsimd.dma_start(out=bcomb[:], in_=bcomb_e[:])
            bf1 = consts.tile([P, OC], f32)
            nc.gpsimd.dma_start(out=bf1[:], in_=bf1_e[:])
            bf2 = consts.tile([P, OC], f32)
            nc.gpsimd.dma_start(out=bf2[:], in_=bf2_e[:])
            ident = consts.tile([P, P], bf16)
            nc.gpsimd.dma_start(out=ident[:], in_=ident_e[:])
            ones_row = consts.tile([1, P], bf16)
            nc.vector.memset(ones_row[:], 1.0)
            ones_col = consts.tile([P, 1], bf16)
            nc.vector.memset(ones_col[:], 1.0)

            # persistent accumulators
            cpool = small.tile([P, KC, BL], f32)       # sum_n cf^T
            outt_sb = small.tile([P, OC, BL], f32)
            if do_pv:
                gp_psum = ps_gp.tile([P, NH, BL], f32)  # sum_n attended
            if do_head:
                cpool_bf = small.tile([P, KC, BL], bf16)
                gp_bf = small.tile([P, NH, BL], bf16)
                comb = small.tile([P, OC, BL], bf16)
                h_sb = small.tile([P, OC, BL], bf16)

            for s in range(n_samples):
                # ---- input DMAs (cast to bf16) ----
                a_t = a_pool.tile([P, KC, N], bf16, tag="a")
                nc.gpsimd.dma_start(out=a_t[:], in_=cft_e[s].rearrange("(k p) n -> p k n", p=P))
                g_t = g_pool.tile([P, KG, N], bf16, tag="g")
                nc.gpsimd.dma_start(out=g_t[:], in_=gft_e[s].rearrange("(k p) n -> p k n", p=P))

                # ---- projections: Q^T, K^T [hd, n] ----
                qt = qk_pool.tile([P, NH, N], bf16, tag="qt")
                kt = qk_pool.tile([P, NH, N], bf16, tag="kt")
                for c2 in range(NH):
                    for nh in range(2):
                        ps = ps_pj.tile([P, 512], mybir.dt.float32, tag="pj")
                        for k in range(KC):
                            nc.tensor.matmul(
                                ps[:], wq[:, k, c2 * P:(c2 + 1) * P],
                                a_t[:, k, nh * 512:(nh + 1) * 512],
                                start=(k == 0), stop=(k == KC - 1))
                        nc.scalar.activation(qt[:, c2, nh * 512:(nh + 1) * 512], ps[:],
                                             AF.Identity, bias=bqk[:, c2:c2 + 1])
                        ps = ps_pj.tile([P, 512], mybir.dt.float32, tag="pj")
                        for k in range(KG):
                            nc.tensor.matmul(
                                ps[:], wk[:, k, c2 * P:(c2 + 1) * P],
                                g_t[:, k, nh * 512:(nh + 1) * 512],
                                start=(k == 0), stop=(k == KG - 1))
                        nc.scalar.activation(kt[:, c2, nh * 512:(nh + 1) * 512], ps[:],
                                             AF.Identity, bias=bqk[:, NH + c2:NH + c2 + 1])

                # ---- V [m, hd] with ones column ----
                v_t = v_pool.tile([P, NT, HD + 1], bf16, tag="v")
                nc.vector.memset(v_t[:, :, HD:HD + 1], 1.0)
                for mt in range(NT):
                    ps = ps_pj.tile([P, 512], mybir.dt.float32, tag="pj")
                    for k in range(KG):
                        nc.tensor.matmul(ps[:, :HD], g_t[:, k, mt * P:(mt + 1) * P],
                                         wv[:, k, :], start=(k == 0), stop=False)
                    nc.tensor.matmul(ps[:, :HD], ones_row[:1, :],
                                     bv[:], start=False, stop=True)
                    nc.scalar.copy(v_t[:, mt, :HD], ps[:, :HD])

                # ---- cluster pool: sum_n cf^T (free-axis reduce) ----
                for k in range(KC):
                    nc.vector.reduce_sum(cpool[:, k, s:s + 1], a_t[:, k, :],
                                         axis=mybir.AxisListType.X)

                # ---- scores -> E -> P per row-tile ----
                p_t = p_pool.tile([P, NT, N], bf16, tag="p")
                rl = small.tile([P, NT], f32, tag=f"rl{s}")
                for nt in range(NT):
                    cw_t = cw_pool.tile([P, N], bf16, tag="cw")
                    nc.gpsimd.dma_start(out=cw_t[:], in_=cw_e[s, nt * P:(nt + 1) * P, :])
                    ps_s = ps_sc.tile([P, N], mybir.dt.float32, tag="sc")
                    for mh in range(2):
                        for c2 in range(NH):
                            nc.tensor.matmul(
                                ps_s[:, mh * 512:(mh + 1) * 512],
                                qt[:, c2, nt * P:(nt + 1) * P],
                                kt[:, c2, mh * 512:(mh + 1) * 512],
                                start=(c2 == 0), stop=(c2 == NH - 1))
                    e_t = e_pool.tile([P, N], bf16, tag="e")
                    nc.scalar.activation(e_t[:], ps_s[:], AF.Exp, scale=float(1.0 / np.sqrt(HD)))
                    nc.vector.tensor_mul(p_t[:, nt, :], e_t[:], cw_t[:])

                if not do_pv:
                    nc.gpsimd.dma_start(out=attn_e[s].rearrange("(t p) m -> p t m", p=P),
                                        in_=p_t[:])
                    continue
                # ---- P^T via PE transpose ----
                pt_t = pt_pool.tile([P, NT, N], bf16, tag="ptt")
                for mt in range(NT):
                    ps_t = ps_pt.tile([P, N], bf16, tag="pt")
                    for nt in range(NT):
                        nc.tensor.transpose(ps_t[:, nt * P:(nt + 1) * P],
                                            p_t[:, nt, mt * P:(mt + 1) * P], ident[:])
                    nc.vector.tensor_copy(pt_t[:, mt, :], ps_t[:])

                # ---- attended + row-sums; normalize ----
                att_t = att_pool.tile([P, NT, HD], bf16, tag="att")
                for nt in range(NT):
                    ps_a = ps_av.tile([P, 512], mybir.dt.float32, tag="av")
                    for mt in range(NT):
                        nc.tensor.matmul(ps_a[:, :HD + 1], pt_t[:, mt, nt * P:(nt + 1) * P],
                                         v_t[:, mt, :], start=(mt == 0), stop=(mt == NT - 1))
                    nc.vector.reciprocal(rl[:, nt:nt + 1], ps_a[:, HD:HD + 1])
                    nc.vector.tensor_scalar_mul(att_t[:, nt, :], in0=ps_a[:, :HD],
                                                scalar1=rl[:, nt:nt + 1])
                    nc.vector.tensor_scalar_mul(p_t[:, nt, :], in0=p_t[:, nt, :],
                                                scalar1=rl[:, nt:nt + 1])
                if do_attn_out:
                    nc.gpsimd.dma_start(out=attn_e[s].rearrange("(t p) m -> p t m", p=P),
                                        in_=p_t[:])

                # ---- geom pool: sum_n attended via ones-matmul ----
                for c2 in range(NH):
                    for nt in range(NT):
                        nc.tensor.matmul(gp_psum[:, c2, s:s + 1],
                                         att_t[:, nt, c2 * P:(c2 + 1) * P], ones_col[:],
                                         start=(nt == 0), stop=(nt == NT - 1))

            if not do_head:
                nc.vector.tensor_copy(outt_sb[:, 0, :], cpool[:, 0, :])
                head_iter = []
            else:
                head_iter = [0]
            # ---- pooled projections -> combined ----
            for _ in head_iter:
              nc.vector.tensor_copy(cpool_bf[:], cpool[:])
              nc.vector.tensor_copy(gp_bf[:], gp_psum[:])
            for m5 in (range(O2) if do_head else []):
                ps = ps_pj.tile([P, 512], mybir.dt.float32, tag="pj")
                for k in range(KC):
                    nc.tensor.matmul(ps[:, :BL], wcp[:, k, m5 * P:(m5 + 1) * P],
                                     cpool_bf[:, k, :], start=(k == 0), stop=(k == KC - 1))
                nc.scalar.activation(comb[:, m5, :], ps[:, :BL], AF.Identity,
                                     bias=bcomb[:, m5:m5 + 1])
                ps = ps_pj.tile([P, 512], mybir.dt.float32, tag="pj")
                for k in range(NH):
                    nc.tensor.matmul(ps[:, :BL], wgp[:, k, m5 * P:(m5 + 1) * P],
                                     gp_bf[:, k, :], start=(k == 0), stop=(k == NH - 1))
                nc.scalar.activation(comb[:, O2 + m5, :], ps[:, :BL], AF.Identity,
                                     bias=bcomb[:, O2 + m5:O2 + m5 + 1])

            # ---- head: h = relu(bn(comb @ Wf1 + bf1)) ; out = h @ Wf2 + bf2 ----
            for m10 in (range(OC) if do_head else []):
                ps = ps_pj.tile([P, 512], mybir.dt.float32, tag="pj")
                for k in range(OC):
                    nc.tensor.matmul(ps[:, :BL], wf1[:, k, m10 * P:(m10 + 1) * P],
                                     comb[:, k, :], start=(k == 0), stop=(k == OC - 1))
                nc.scalar.activation(h_sb[:, m10, :], ps[:, :BL], AF.Relu,
                                     bias=bf1[:, m10:m10 + 1])
            for m10 in (range(OC) if do_head else []):
                ps = ps_pj.tile([P, 512], mybir.dt.float32, tag="pj")
                for k in range(OC):
                    nc.tensor.matmul(ps[:, :BL], wf2[:, k, m10 * P:(m10 + 1) * P],
                                     h_sb[:, k, :], start=(k == 0), stop=(k == OC - 1))
                nc.scalar.activation(outt_sb[:, m10, :], ps[:, :BL], AF.Identity,
                                     bias=bf2[:, m10:m10 + 1])
            nc.gpsimd.dma_start(out=outt_e.rearrange("(c p) s -> p c s", p=P),
                                in_=outt_sb[:])

    if split:
        _split_waits(nc, mybir)
    return nc


def _marshal(inputs):
    f = np.float32
    cf = np.asarray(inputs["cluster_features"], f)
    gf = np.asarray(inputs["geometric_features"], f)
    cw = np.asarray(inputs["cluster_weights"], f)
    Wq, bq = np.asarray(inputs["Wq"], f), np.asarray(inputs["bq"], f)
    Wk, bk = np.asarray(inputs["Wk"], f), np.asarray(inputs["bk"], f)
    Wv, bv = np.asarray(inputs["Wv"], f), np.asarray(inputs["bv"], f)
    Wcp, bcp = np.asarray(inputs["Wcp"], f), np.asarray(inputs["bcp"], f)
    Wgp, bgp = np.asarray(inputs["Wgp"], f), np.asarray(inputs["bgp"], f)
    Wf1, bf1 = np.asarray(inputs["Wf1"], f), np.asarray(inputs["bf1"], f)
    Wf2, bf2 = np.asarray(inputs["Wf2"], f), np.asarray(inputs["bf2"], f)
    g_, b_ = np.asarray(inputs["bn_gamma"], f), np.asarray(inputs["bn_beta"], f)
    mu, var = np.asarray(inputs["bn_mean"], f), np.asarray(inputs["bn_var"], f)

    s = (g_ / np.sqrt(var + EPS_BN)).astype(f)
    wf1_eff = (Wf1 * s[None, :]).astype(f)
    bf1_eff = ((bf1 - mu) * s + b_).astype(f)

    pc = lambda b, c: np.ascontiguousarray(b.reshape(c, P).T)
    shared = {
        "wq": Wq, "wk": Wk, "wv": Wv,
        "wcp": (Wcp / N).astype(f), "wgp": (Wgp / N).astype(f),
        "wf1": wf1_eff, "wf2": Wf2,
        "bqk": np.concatenate([pc(bq, NH), pc(bk, NH)], axis=1),
        "bv": np.ascontiguousarray(bv[None, :]),
        "bcomb": pc(np.concatenate([bcp, bgp]), OC),
        "bf1": pc(bf1_eff, OC), "bf2": pc(bf2, OC),
        "ident": np.eye(P, dtype=f),
    }
    in_maps = []
    for c in range(N_CORES):
        sl = slice(c * BL, (c + 1) * BL)
        m = dict(shared)
        m["cft"] = np.ascontiguousarray(cf[sl].transpose(0, 2, 1))
        m["gft"] = np.ascontiguousarray(gf[sl].transpose(0, 2, 1))
        m["cw"] = np.ascontiguousarray(cw[sl])
        in_maps.append(m)
    return in_maps


def kernel(**inputs):
    from concourse.bass_utils import run_bass_kernel_spmd
    if "nc" not in _CACHE:
        _CACHE["nc"] = _build()
    nc = _CACHE["nc"]
    in_maps = _marshal(inputs)
    res = run_bass_kernel_spmd(nc, in_maps, list(range(N_CORES)), trace=False)
    out = np.concatenate([np.ascontiguousarray(r["outt"].T) for r in res.results], axis=0)
    attn = np.concatenate([r["attn"] for r in res.results], axis=0)
    return out.astype(np.float32), attn.astype(np.float32)
